# revision 52
# baseline (speedup 1.0000x reference)
"""Trainium2 Bass kernel for nn_MultiHeadAttention (B=2, S=4096, D=512, H=8).

Computes: q/k/v = relu(x@W+b) per head, softmax(q k^T / sqrt(64)) v,
out = relu(concat_heads @ Wo + bo).

Sharding: 8 cores = 2 (batch) x 4 (query-slice).  Each core computes full
K/V projections for its batch (redundant across the 4 q-slice cores) and
attention + output projection for its 1024-row query slice.  No collectives;
the host concatenates the 8 output slices.

v2 pipeline (vs v1 ~288us -> ~260us, rel err 0.0098 -> 0.0029):
- Unified U-lagged stages: every head-pair block's probabilities are
  written to a full 32-ktile fp8 buffer (two ping-pong buffers); stage s
  runs block s's QK+exp while the PE consumes block s-1's buffer with
  DoubleRow U matmuls.  No per-block drain, no separate prefetch paths.
- PSUM (8 banks): "scores" pair-tiles [P,2,512] x bufs=3 (3 ktiles in
  flight, vs v1's 2, loosening the exp -> scores-slot recycling chain)
  + "psU" x 1.  Projections and output chains BORROW scores-pool slots;
  there is no separate proj bank.  NOTE: dependency tracking here is
  effectively whole-tile — slot rotation must go through the tile-pool
  tag machinery; slicing one big PSUM tile serializes everything (that
  variant measured 566us), and an SBUF tile written by consecutive
  stages creates false stage-boundary WARs (hence per-stage buffers).
- Greedy ACT/DVE balancer assigns every exp and every PSUM eviction to
  whichever engine has less projected busy-time (exp: ACT exact table
  exp vs DVE Schraudolph int8 bit-trick; evictions: ACT activation
  relu+bias vs DVE tensor_scalar).  More exps land on the exact ACT
  path than v1's fixed 17/32 split, which is where the error win
  comes from.
- finish_block: ACT ln+exp reciprocal of the denominator row (DVE
  reciprocal_approx_fast reading PSUM returns garbage — measured),
  gpsimd partition_broadcast, then the normalize multiplies read U
  straight out of PSUM (drops v1's copy).
- psU is TWO single-bank tiles (one per head): with one [P,2,512] tile,
  whole-tile tracking made every stage's first U matmul wait the whole
  previous finish chain; per-head tiles cut ~14us.  The final stage
  runs all head-a U passes, head-a's finish, then head-b's — so the
  only serial tail is one head's ln/exp/broadcast/normalize (~3.8us).
- kproj fillers run INSIDE their own consuming stage (chunk n covers
  ktiles 4n..4n+3 while the scores stream sits at ~2tp+5, so evictions
  land ahead of their consumers); PE warmup matmuls open the p-state
  ramp window before the first projection; gate mini-matmuls removed
  (whole-tile OT tracking already orders output chains).
- Device note: back-to-back runs heat/power-throttle the PE (~15-20%
  swings; 274us cold vs 330us hot).  Benchmark after ~2min idle.
"""

import numpy as np
import ml_dtypes

import concourse.bass as bass
import concourse.mybir as mybir
import concourse.tile as tile
from concourse import bacc
from concourse import bass_utils
from concourse import hw_specs


def _patch_act_tables():
    """Make exp/relu/ln all resolve to the one table set that contains all
    three (natural_log_exp_and_others).  The load-insertion pass assigns
    each ACTIVATE the *first* set containing its function, so a kernel
    mixing exp and ln otherwise reloads tables around every ln (~2.7us per
    switch).  Only set *selection* changes; set contents seen by the
    runtime are untouched."""
    if getattr(hw_specs, "_mha_act_patch", False):
        return
    orig = hw_specs.get_activation_tables
    HOME = "natural_log_exp_and_others"
    AF_ = mybir.ActivationFunctionType

    def patched(arch):
        tables = orig(arch)
        if HOME not in tables:
            return tables
        out = {}
        for name, funcs in tables.items():
            if name != HOME:
                funcs = funcs - {AF_.Exp, AF_.Relu, AF_.Ln}
            out[name] = funcs
        return out

    hw_specs.get_activation_tables = patched
    bacc.get_activation_tables = patched
    hw_specs._mha_act_patch = True

F32 = mybir.dt.float32
BF16 = mybir.dt.bfloat16
FP8 = mybir.dt.float8e4
I8 = mybir.dt.int8
AF = mybir.ActivationFunctionType
ALU = mybir.AluOpType
DR = mybir.MatmulPerfMode.DoubleRow

P = 128
D = 512
H = 8
DH = 64
DT = D // P  # 4 (also = number of head pairs)
B = 2
S = 4096
NCORES = 8
QSPLIT = 4
SQ_FULL = S // QSPLIT  # 1024 query rows per core
QC = 512               # q-chunk (matmul free dim / PSUM bank width)
VP = 80                # padded V row stride (65 used; 80 keeps fp8 16B align)
NSLOT = 5              # single-bank PSUM score slots

# exp folding: pT = exp(s/8 + EXPB); the e^EXPB factor cancels in normalize.
EXPB = -2.9
LOG2E = 1.4426950408889634
# DVE bit-trick: int8(round(s*A8 + B8)) bits == fp8e4(exp(s/8 + EXPB))
A8 = (1 << 3) * LOG2E / 8.0
C8 = 0.35
B8 = 7 * (1 << 3) + (1 << 3) * LOG2E * EXPB - C8

# balancer cost constants (us of engine busy time)
C_EXP_A, C_EXP_D = 1.04, 1.15      # [128,1024] exp
C_EXPH_A, C_EXPH_D = 0.61, 0.66    # [128,512] half exp
C_EV_A, C_EV_D = 0.61, 0.66        # [128,512] PSUM->SBUF eviction
C_RCP_D = 0.80                     # [1,512] reciprocal_approx_fast
C_LN_A = 1.25                      # [1,512] ln + exp pair on ACT
C_TT_D = 0.70                      # [64,512] normalize TT from PSUM


def build_mha(sk=S, sq=SQ_FULL, skip_vbias=False, skip_obias=False):
    """Build the SPMD Bass program (identical on all cores).

    All inputs arrive pre-tiled by the host into exact SBUF layout
    ([128 partitions, contiguous free bytes]) so every load is a max-packet
    linear DMA."""
    _patch_act_tables()
    nc = bacc.Bacc("TRN2", target_bir_lowering=False, debug=False,
                   num_devices=NCORES)

    xT_d = nc.dram_tensor("xT_f8", (P, DT * sk), FP8,
                          kind="ExternalInput").ap()  # chunk-major, see prep
    xqT_d = nc.dram_tensor("xqT_f8", (P, DT * sq), FP8,
                           kind="ExternalInput").ap()
    w_dram = {}
    for n in ("wq", "wk", "wv"):
        w_dram[n] = nc.dram_tensor(n, (P, DT * D), FP8,
                                   kind="ExternalInput").ap()
    w_dram["wo"] = nc.dram_tensor("wo", (P, DT * D), BF16,
                                  kind="ExternalInput").ap()
    b_dram = {
        "bq": nc.dram_tensor("bq", (P, DT), F32, kind="ExternalInput").ap(),
        "bk": nc.dram_tensor("bk", (P, DT), F32, kind="ExternalInput").ap(),
        "bv": nc.dram_tensor("bv", (1, D), BF16, kind="ExternalInput").ap(),
        "bo": nc.dram_tensor("bo", (1, D), BF16, kind="ExternalInput").ap(),
    }
    out = nc.dram_tensor("out", (sq, D), F32, kind="ExternalOutput").ap()

    with tile.TileContext(nc) as tc:
        _build_tile(tc, xT_d, xqT_d, w_dram, b_dram, out, sk, sq,
                    skip_vbias, skip_obias)

    nc.compile()
    return nc


def _build_tile(tc, xT_d, xqT_d, w_dram, b_dram, out, sk, sq,
                skip_vbias=False, skip_obias=False):
    nc = tc.nc
    SK_T = sk // P            # ktiles of the key/value sequence (32)
    NKTP = SK_T // 2          # ktile pairs per head (16)
    SQ_T = sq // P            # 8
    NQC = sq // QC            # q chunks per core (2)
    CH = min(4, SK_T)         # stiles per x chunk
    NCH = SK_T // CH          # 8
    assert NQC == 2 and SK_T == 32

    # greedy ACT/DVE busy-time balancer.  ACT is seeded with its two
    # activation-table loads.
    eng_t = {"a": 2.6, "d": 0.0}

    def pick(ca, cd):
        e = "a" if eng_t["a"] + ca <= eng_t["d"] + cd else "d"
        eng_t[e] += ca if e == "a" else cd
        return e

    with (
        tc.tile_pool(name="singles", bufs=1) as singles,
        tc.tile_pool(name="work", bufs=3) as work,
        tc.tile_pool(name="pmain", bufs=1, space="PSUM") as pmain,
    ):
        # ---- startup: only what Q-proj pair 0 needs, first ----
        w_bf = {}
        w_bf["wq"] = singles.tile([P, DT, D], FP8, name="wq_f8")
        wq_src = w_dram["wq"].rearrange("p (t n) -> p t n", t=DT)
        nc.sync.dma_start(w_bf["wq"][:, 0:2], wq_src[:, 0:2])
        xTq = singles.tile([P, DT, sq], FP8)
        xTq_src = xqT_d.rearrange("p (t s) -> p t s", t=DT)
        # split so qproj(0,0)'s first matmul (q cols 0:512) starts sooner
        nc.scalar.dma_start(xTq[:, 0:2, 0:QC], xTq_src[:, 0:2, 0:QC])
        nc.scalar.dma_start(xTq[:, 0:2, QC:sq], xTq_src[:, 0:2, QC:sq])
        b_col = {}
        b_col["bq"] = singles.tile([P, DT], F32, name="bq_col")
        # x chunk 0 ahead of wq's second half: kproj/vproj start earlier
        # while qproj's first two matmuls cover the wq23 delay.
        CHP = CH * P
        xT = singles.tile([P, NCH, DT, CHP], FP8)
        xT_src = xT_d.rearrange("p (n t s) -> p n t s", n=NCH, t=DT)
        nc.sync.dma_start(xT[:, 0], xT_src[:, 0])
        nc.sync.dma_start(w_bf["wq"][:, 2:4], wq_src[:, 2:4])
        nc.scalar.dma_start(xTq[:, 2:4], xTq_src[:, 2:4])
        nc.scalar.dma_start(b_col["bq"], b_dram["bq"])

        # ---- K/V/O-proj deps next, spread across DMA queues so the
        # chunk phase's x tile and wk/wv aren't head-of-line blocked ----
        w_bf["wk"] = singles.tile([P, DT, D], FP8, name="wk_f8")
        nc.gpsimd.dma_start(w_bf["wk"], w_dram["wk"].rearrange(
            "p (t n) -> p t n", t=DT))
        b_col["bk"] = singles.tile([P, DT], F32, name="bk_col")
        nc.scalar.dma_start(b_col["bk"], b_dram["bk"])
        nc.sync.dma_start(xT[:, 1], xT_src[:, 1])
        b_row = {}
        for n in ("wv", "wo"):
            dt_n = BF16 if n == "wo" else FP8
            wb = singles.tile([P, DT, D], dt_n, name=f"{n}_w")
            (nc.gpsimd if n == "wv" else nc.sync).dma_start(
                wb, w_dram[n].rearrange("p (t n) -> p t n", t=DT))
            w_bf[n] = wb
            if n == "wv" and not skip_vbias:
                br = singles.tile([1, D], BF16, name="bv_row")
                nc.sync.dma_start(br, b_dram["bv"])
                b_row["bv"] = br
        if not skip_obias:
            br = singles.tile([1, D], BF16, name="bo_row")
            nc.sync.dma_start(br, b_dram["bo"])
            b_row["bo"] = br

        # ---- persistent SBUF tensors ----
        bias_t = singles.tile([P, 1], F32)
        nc.vector.memset(bias_t, EXPB)
        xT1 = None
        if not skip_vbias:
            xT1 = singles.tile([1, sk], BF16)
            nc.vector.memset(xT1, 1.0)
        KT = singles.tile([P, DT, sk], BF16)
        V_pad = singles.tile([P, NKTP, H, 2, VP], FP8)
        nc.vector.memset(V_pad[:, :, :, :, DH:DH + 1], 1.0)
        QT = singles.tile([P, DT, sq], BF16)
        OT = singles.tile([P, DT, sq], BF16)
        OT1 = singles.tile([1, P], BF16)    # ones row: obias-gate stationary
        nc.vector.memset(OT1, 1.0)
        OTG = singles.tile([1, SQ_T], BF16)  # per-qt ordering-gate columns
        nc.vector.memset(OTG, 1.0)
        # ping-pong full-block probability buffers
        PT_A = singles.tile([P, SK_T, 2, QC], FP8, name="PT_A")
        PT_B = singles.tile([P, SK_T, 2, QC], FP8, name="PT_B")
        pbufs = [PT_A, PT_B]

        # 5-slot single-bank score ring: ktile k -> banks (2k)%5, (2k+1)%5
        # PSUM layout (8 banks): "scores" pair-tiles [P,2,512] x bufs=3
        # (6 banks; 3 ktiles in flight so the exp->scores recycling chain
        # never gates the exp engines) + "psU" [P,2,512] x 1 (2 banks).
        # Projections and output chains BORROW scores slots (pool-rotation
        # WAR keeps them race-free); there is no separate proj bank.
        def score_pair():
            return pmain.tile([P, 2, QC], F32, tag="scores", bufs=3,
                              name="psS")

        def proj_tile():
            return score_pair()[:, 0, :]
        def evict(dst, src, bias=None):
            """Relu (+bias) PSUM->SBUF eviction on whichever engine is
            less loaded."""
            if pick(C_EV_A, C_EV_D) == "a":
                if bias is None:
                    nc.scalar.activation(dst, src, AF.Relu)
                else:
                    nc.scalar.activation(dst, src, AF.Relu, bias=bias)
            else:
                if bias is None:
                    nc.vector.tensor_scalar(dst, src, 0.0, None, op0=ALU.max)
                else:
                    nc.vector.tensor_scalar(dst, src, bias, 0.0,
                                            op0=ALU.add, op1=ALU.max)

        def exp_emit(dst_f, src_f, eng):
            if eng == "a":
                nc.scalar.activation(dst_f, src_f, AF.Exp, scale=0.125,
                                     bias=bias_t)
            else:
                nc.vector.tensor_scalar(dst_f.bitcast(I8), src_f, A8, B8,
                                        op0=ALU.mult, op1=ALU.add)

        def qk1(j, qc, kt, dstbuf):
            """Scores + exp for BOTH heads of pair j at ktile kt.  The two
            QK matmuls sit in different PE row groups (partitions 0-63 vs
            64-127) and run concurrently; one 1024-wide exp covers both
            heads."""
            psS = score_pair()
            q0 = qc * QC
            for a in (0, 1):
                h0 = a * DH
                nc.tensor.matmul(
                    psS[:, a, :],
                    KT[h0:h0 + DH, j, kt * P:(kt + 1) * P],
                    QT[h0:h0 + DH, j, q0:q0 + QC], start=True, stop=True)
            dst = dstbuf[:, kt]  # [P, 2, QC]
            exp_emit(dst.rearrange("p a b -> p (a b)"),
                     psS.rearrange("p a b -> p (a b)"),
                     pick(C_EXP_A, C_EXP_D))

        def u_one(j, tp, pt, a, buf):
            nc.tensor.matmul(
                pt, V_pad[:, tp, 2 * j + a, :, 0:DH + 1],
                buf[:, 2 * tp:2 * tp + 2, a, :],
                start=(tp == 0), stop=(tp == NKTP - 1), perf_mode=DR)

        def u_pair(j, tp, psU, buf):
            """DoubleRow U matmuls for both heads of ktile pair tp, reading
            the block probability buffer (slot stride 2*QC fp8)."""
            for a in (0, 1):
                u_one(j, tp, psU[a], a, buf)

        def finish_head(j, qc, psU, a):
            """One head's U done: reciprocal of the denominator row
            (accumulated by V's ones column at partition DH), gpsimd
            broadcast, then normalize straight out of PSUM into OT."""
            q0 = qc * QC
            if True:
                h0 = a * DH
                rcp = work.tile([1, QC], F32, tag="rcp", bufs=2, name="rcp")
                den = psU[a][DH:DH + 1, :]
                if False and pick(C_LN_A, C_RCP_D) == "d":
                    nc.vector.reciprocal_approx_fast(rcp, den)
                else:
                    eng_t["a"] += C_LN_A
                    lnd = work.tile([1, QC], F32, tag="lnd", bufs=2,
                                    name="lnd")
                    nc.scalar.activation(lnd, den, AF.Ln)
                    nc.scalar.activation(rcp, lnd, AF.Exp, scale=-1.0)
                brc = work.tile([DH, QC], F32, tag="brc", bufs=2, name="brc")
                nc.gpsimd.partition_broadcast(brc, rcp)
                eng_t["d"] += C_TT_D
                nc.vector.tensor_mul(
                    OT[h0:h0 + DH, j, q0:q0 + QC], psU[a][0:DH, :], brc)

        def finish_block(j, qc, psU):
            finish_head(j, qc, psU, 0)
            finish_head(j, qc, psU, 1)

        def gate(j, qc, qts):
            """Tiny rewrite of OTG columns that depends on block (j, qc)'s
            normalize writes — gates outproj chains (whose bias mini-matmul
            streams the OTG column) behind the last normalize.  Two chained
            ops (WAW) since TensorTensor inputs must share a start
            partition; the written values are never meaningfully consumed."""
            q0 = qc * QC
            n = len(qts)
            for h0 in (0, DH):
                nc.vector.tensor_tensor(
                    OTG[0:1, qts[0]:qts[0] + n],
                    OT[h0:h0 + 1, j, q0:q0 + n],
                    OT[h0:h0 + 1, j, q0 + n:q0 + 2 * n], op=ALU.max)

        def obias_mm(psO, qt):
            """Ordering gate (+ bias when bo!=0).  skip_obias: stream the
            OTG gate column through a ones stationary (~60 cycles); the j=0
            weight matmul then clears PSUM with start=True."""
            if skip_obias:
                # whole-tile OT dependency tracking already orders the
                # weight matmuls behind every normalize they read; no
                # gate mini-matmul needed.
                return True
            nc.tensor.matmul(psO, OT1, b_row["bo"], start=True, stop=False)
            return False

        def out_dma(qt, o_sb):
            nc.sync.dma_start(out[qt * P:(qt + 1) * P, :], o_sb)

        def outproj(qt, pool_tile):
            psO = pool_tile()
            restart = obias_mm(psO, qt)
            for j in range(DT):
                nc.tensor.matmul(psO, OT[:, j, qt * P:(qt + 1) * P],
                                 w_bf["wo"][:, j, :],
                                 start=(restart and j == 0),
                                 stop=(j == DT - 1))
            o_sb = work.tile([P, D], F32, tag="osb", bufs=2, name="o_sb")
            evict(o_sb, psO)
            out_dma(qt, o_sb)

        def qproj(j, nq, ptile):
            psQ = ptile()
            for t2 in range(DT // 2):
                nc.tensor.matmul(
                    psQ,
                    w_bf["wq"][:, 2 * t2:2 * t2 + 2, j * P:(j + 1) * P],
                    xTq[:, 2 * t2:2 * t2 + 2, nq * QC:(nq + 1) * QC],
                    start=(t2 == 0), stop=(t2 == DT // 2 - 1),
                    perf_mode=DR)
            evict(QT[:, j, nq * QC:(nq + 1) * QC], psQ,
                  bias=b_col["bq"][:, j:j + 1])

        def kproj(j, n, ptile):
            psK = ptile()
            for t2 in range(DT // 2):
                nc.tensor.matmul(
                    psK, w_bf["wk"][:, 2 * t2:2 * t2 + 2,
                                    j * P:(j + 1) * P],
                    xT[:, n, 2 * t2:2 * t2 + 2, :],
                    start=(t2 == 0), stop=(t2 == DT // 2 - 1),
                    perf_mode=DR)
            evict(KT[:, j, n * CHP:(n + 1) * CHP], psK,
                  bias=b_col["bk"][:, j:j + 1])

        def vproj(st, ptile):
            n, si = st // CH, st % CH
            psV = ptile()
            for t2 in range(DT // 2):
                nc.tensor.matmul(
                    psV,
                    xT[:, n, 2 * t2:2 * t2 + 2, si * P:(si + 1) * P],
                    w_bf["wv"][:, 2 * t2:2 * t2 + 2, :],
                    start=(t2 == 0),
                    stop=(skip_vbias and t2 == DT // 2 - 1),
                    perf_mode=DR)
            if not skip_vbias:
                nc.tensor.matmul(psV, xT1[:, st * P:(st + 1) * P],
                                 b_row["bv"], start=False, stop=True)
            evict(V_pad[:, st // 2, :, st % 2, 0:DH],
                  psV.rearrange("p (h d) -> p h d", h=H))

        # ================= chunk phase =================
        # x load + V proj + K proj pairs 0,1 + block (0,0) scores+exp;
        # projections borrow scores-pool slots between qk1 allocations.
        # PE p-state warmup: tiny matmuls gated only on bias_t's memset
        # start the ~3.4us ramp window before the first projection stream.
        for _ in range(2):
            psw = score_pair()
            nc.tensor.matmul(psw[0:1, 0, 0:1], bias_t[0:1], bias_t[0:1],
                             start=True, stop=True)
        qproj(0, 0, proj_tile)
        qproj(0, 1, proj_tile)
        for n in range(NCH):
            if 1 <= n + 2 and n + 2 < NCH:
                nc.sync.dma_start(xT[:, n + 2], xT_src[:, n + 2])
            kproj(0, n, proj_tile)
            if n <= 1:
                kproj(1, n, proj_tile)
            for i in range(CH):
                vproj(n * CH + i, proj_tile)
                qk1(0, 0, n * CH + i, PT_A)
            if n == 2:
                qproj(1, 0, proj_tile)
            if n == 4:
                qproj(1, 1, proj_tile)

        # ================= stages =================
        # stage si: U matmuls over block si-1's buffer + scores/exp of
        # block si into the other buffer; deferred projections and output
        # rows slot in as fillers.
        blocks = [(0, 0), (1, 0), (2, 0), (3, 0),
                  (0, 1), (1, 1), (2, 1), (3, 1)]

        def new_psU():
            # two single-bank tiles (one per head) so one head's finish
            # chain creates no false whole-tile WAR on the other's U.
            return (pmain.tile([P, QC], F32, tag="psUa", bufs=1,
                               name="psUa")[0:DH + 1],
                    pmain.tile([P, QC], F32, tag="psUb", bufs=1,
                               name="psUb")[0:DH + 1])

        # stage fillers.  "early" fillers run one-per-tp from the stage
        # start: kproj chunks for the STAGE'S OWN block — chunk n covers
        # ktiles 4n..4n+3 while the scores stream sits at ktile ~2tp+5,
        # so the eviction always lands ahead of its consumers.  "late"
        # fillers (next block's projections, output rows) space out over
        # the rest of the stage.
        early = {si: [] for si in range(1, 8)}
        late = {si: [] for si in range(1, 8)}
        for n in range(2, NCH):
            early[1].append(lambda n=n: kproj(1, n, proj_tile))
        for n in range(4, NCH):
            early[2].append(lambda n=n: kproj(2, n, proj_tile))
        for n in range(2, NCH):
            early[3].append(lambda n=n: kproj(3, n, proj_tile))
        late[1] += [lambda: qproj(2, 0, proj_tile),
                    lambda: qproj(2, 1, proj_tile)]
        late[1] += [lambda n=n: kproj(2, n, proj_tile) for n in range(4)]
        late[2] += [lambda: qproj(3, 0, proj_tile),
                    lambda: qproj(3, 1, proj_tile)]
        late[2] += [lambda n=n: kproj(3, n, proj_tile) for n in range(2)]
        late[5] += [lambda: outproj(0, proj_tile),
                    lambda: outproj(1, proj_tile)]
        late[6].append(lambda: outproj(2, proj_tile))
        late[7].append(lambda: outproj(3, proj_tile))

        for si in range(1, 8):
            jc, qcc = blocks[si]
            jp, qcp = blocks[si - 1]
            cur = pbufs[si % 2]
            prev = pbufs[(si - 1) % 2]
            psU = new_psU()
            el, fl = early[si], late[si]
            fi = 0
            nrest = NKTP - len(el) - 2
            spacing = max(1, nrest // (len(fl) + 1)) if fl else 99
            for tp in range(NKTP):
                qk1(jc, qcc, 2 * tp, cur)
                qk1(jc, qcc, 2 * tp + 1, cur)
                if tp < len(el):
                    el[tp]()
                elif fl and fi < len(fl) and \
                        (tp - len(el) + 1) % spacing == 0:
                    fl[fi]()
                    fi += 1
                u_pair(jp, tp, psU, prev)
            while fi < len(fl):
                fl[fi]()
                fi += 1
            finish_block(jp, qcp, psU)
            if si == 4 and not skip_obias:
                gate(3, 0, [0, 1, 2, 3])

        # ---- final stage: U over block (3,1) + open output chains ----
        psU = new_psU()
        open_psO = []

        def open_chain(qt, ptile=proj_tile):
            """Partial outproj chain (gate + first DT-1 weight tiles) in a
            borrowed scores slot; the final tile waits the last normalize."""
            psO = ptile()
            restart = obias_mm(psO, qt)
            for j in range(DT - 1):
                nc.tensor.matmul(psO, OT[:, j, qt * P:(qt + 1) * P],
                                 w_bf["wo"][:, j, :],
                                 start=(restart and j == 0), stop=False)
            open_psO.append((qt, psO))

        qt_lo = SQ_T // NQC
        fbuf = pbufs[7 % 2]
        for tp in range(NKTP):
            u_one(3, tp, psU[0], 0, fbuf)
            if tp == 10:
                open_chain(qt_lo)
            if tp == 13:
                open_chain(qt_lo + 1)
        finish_head(3, 1, psU, 0)
        for tp in range(NKTP):
            u_one(3, tp, psU[1], 1, fbuf)
            if tp == 3:
                open_chain(qt_lo + 2)
            if tp == 8:
                open_chain(qt_lo + 3,
                           lambda: pmain.tile([P, QC], F32, tag="psUa",
                                              bufs=1, name="psO4"))
        finish_head(3, 1, psU, 1)
        if not skip_obias:
            gate(3, 1, [qt_lo, qt_lo + 1, qt_lo + 2, qt_lo + 3])
        # tail: evictions pinned to ACT (free after the last ln/exp; the
        # DVE queue is head-of-line blocked here), own buffer ring so no
        # eviction waits an earlier row's DMA read.
        for qt, psO in open_psO:
            nc.tensor.matmul(psO, OT[:, DT - 1, qt * P:(qt + 1) * P],
                             w_bf["wo"][:, DT - 1, :],
                             start=False, stop=True)
            o_sb = work.tile([P, D], F32, tag="osbt", bufs=4, name="o_sbt")
            nc.scalar.activation(o_sb, psO, AF.Relu)
            nc.sync.dma_start(out[qt * P:(qt + 1) * P, :], o_sb)


_NC_CACHE = {}


def _get_nc(sk=S, sq=SQ_FULL, skip_vbias=False, skip_obias=False):
    key = (sk, sq, skip_vbias, skip_obias)
    if key not in _NC_CACHE:
        _NC_CACHE[key] = build_mha(sk, sq, skip_vbias, skip_obias)
    return _NC_CACHE[key]


def _tile_rows(a):
    """[D, n] -> SBUF layout [P, DT*n]: partition p gets rows p, 128+p, ..."""
    Dd, n = a.shape
    t = Dd // P
    return np.ascontiguousarray(
        a.reshape(t, P, n).transpose(1, 0, 2).reshape(P, t * n))


def _tile_chunks(a, chp):
    """[D, sk] -> chunk-major SBUF layout [P, NCH*DT*chp]: per partition,
    sequence chunks outermost so each chunk is one contiguous linear DMA."""
    Dd, sk = a.shape
    t, nch = Dd // P, sk // chp
    return np.ascontiguousarray(
        a.reshape(t, P, nch, chp).transpose(1, 2, 0, 3).reshape(P, -1))


def prep_inputs(x, Wq, bq, Wk, bk, Wv, bv, Wo, bo):
    """Host-side sharding/layout prep: bf16 casts, feature-major transpose,
    SBUF pre-tiling.  Returns the 8 per-core input maps."""
    bf = ml_dtypes.bfloat16
    f8 = ml_dtypes.float8_e4m3
    x = np.asarray(x, dtype=np.float32)
    shared = {
        "wq": _tile_rows(np.asarray(Wq, np.float32).astype(f8)),
        "wk": _tile_rows(np.asarray(Wk, np.float32).astype(f8)),
        "wv": _tile_rows(np.asarray(Wv, np.float32).astype(f8)),
        "wo": _tile_rows(np.asarray(Wo, np.float32).astype(bf)),
        "bq": np.ascontiguousarray(
            np.asarray(bq, np.float32).reshape(DT, P).T),
        "bk": np.ascontiguousarray(
            np.asarray(bk, np.float32).reshape(DT, P).T),
        "bv": np.asarray(bv, np.float32).astype(bf).reshape(1, D),
        "bo": np.asarray(bo, np.float32).astype(bf).reshape(1, D),
    }
    xT_b = [x[b].T.astype(f8) for b in range(B)]
    xT_tiled = [_tile_chunks(xb, 4 * P) for xb in xT_b]
    in_maps = []
    for c in range(NCORES):
        b, qo = divmod(c, QSPLIT)
        m = dict(shared)
        m["xT_f8"] = xT_tiled[b]
        m["xqT_f8"] = _tile_rows(
            xT_b[b][:, qo * SQ_FULL:(qo + 1) * SQ_FULL])
        in_maps.append(m)
    return in_maps


def kernel(x, Wq, bq, Wk, bk, Wv, bv, Wo, bo, **run_kwargs):
    """Full-input entry point: shards across 8 NeuronCores, returns full out."""
    in_maps = prep_inputs(x, Wq, bq, Wk, bk, Wv, bv, Wo, bo)
    nc = _get_nc(skip_vbias=bool(np.all(np.asarray(bv) == 0)),
                 skip_obias=bool(np.all(np.asarray(bo) == 0)))
    res = bass_utils.run_bass_kernel_spmd(
        nc, in_maps, core_ids=list(range(NCORES)), **run_kwargs)
    full = np.empty((B, S, D), np.float32)
    for c in range(NCORES):
        b, qo = divmod(c, QSPLIT)
        full[b, qo * SQ_FULL:(qo + 1) * SQ_FULL] = res.results[c]["out"]
    if run_kwargs:
        return full, res
    return full
